# revision 20
# baseline (speedup 1.0000x reference)
"""Trainium2 Bass kernel for nn_GAT_55344948576482 (GNN message passing).

Sharding: node dimension N=20000 split across 8 NeuronCores (2500 nodes each).
Fully data-parallel SPMD - no collectives. Small weights/tables replicated.

fp16/bf16 pipeline (rel tol 2e-2; measured ~2.3e-3):
  - host packs slab' = [rel*w2 | ent*w3 | maskbias+fc_b+s1 | pad] fp16
    PARTITION-MAJOR (one contiguous DMA segment per partition per block)
  - e-score tile reductions SPLIT 3-WAY by engine rate:
      DVE: one block tensor_reduce over its tile share
      GPSIMD: per-tile half-adds (no accum support), DVE finishes
      ACT: per-tile Copy+accum
  - softmax (no max-subtraction; mask value -300 underflows exp) in
    [(m,t),k] layout via DVE stream_transpose; one exp ACT per block
  - prod' = slab'_rel (.) slab'_ent -> bf16 (3 chunks DVE / 1 GPSIMD)
  - weighted K-sum on PE: aggT += prod'_t.T @ wall_t + residual item*w2w3;
    diag(w2*w3) undone by per-partition ACT scale at PSUM evac
  - final linear transposed, bias+ReLU fused in evac; output [D, N],
    host transposes back
  - software pipeline: front(b) / fin(b-1) / exp(b-2) / es(b-1) /
    w1(b-3) / prod(b-4) / w2(b-4) / back(b-4); same-engine write->read
    pairs separated by independent ops
"""

import sys

sys.path.insert(0, "/opt/trn_rl_repo")

from contextlib import ExitStack

import ml_dtypes
import numpy as np

import concourse.bass as bass
import concourse.tile as tile
from concourse import bacc
from concourse import mybir
from concourse.bass_utils import run_bass_kernel_spmd

F32 = mybir.dt.float32
BF16 = mybir.dt.bfloat16
FP16 = mybir.dt.float16
NPBF = ml_dtypes.bfloat16
NPF16 = np.float16
AF = mybir.ActivationFunctionType
OP = mybir.AluOpType
AX = mybir.AxisListType

N, K, D = 20000, 32, 128
R = 100
N_CORES = 8
ALPHA = 0.2
NEG_BIG = -300.0           # masked-edge bias; exp underflows, fp16-safe
TPB = 32                   # edge-tiles per block (=> 128 nodes per block)
PRODC = 8                  # tiles per prod chunk
TW = 260                   # tile row: 256 rel'|ent' + maskbias + 3 pad

# e-score reduction tile split per 32-tile block (DVE gets the rest)
GPS_RT = 16                # tiles half-added by GPSIMD, then DVE-reduced
ACT_RT = 7                 # tiles via ACT Copy+accum
PROD_DVE_CHUNKS = 3        # of 4 prod chunks per block, how many on DVE


def build_kernel(num_nodes):
    """Build the single-core Bass program for `num_nodes` nodes."""
    E = num_nodes * K
    NT = E // 128                       # number of [128, D] edge tiles
    NB = (NT + TPB - 1) // TPB          # blocks

    nc = bacc.Bacc("TRN2", target_bir_lowering=False, debug=False)

    # pre-scaled rel|ent + maskbias col, partition-major: [128, NT, TW] fp16
    slabd = nc.dram_tensor("slabd", [128, NT, TW], FP16,
                           kind="ExternalInput").ap()
    # per-block pack: [item*w2w3 natural(128) | at_stream(32)]
    s128 = nc.dram_tensor("s128", [NB, 128, 160], BF16,
                          kind="ExternalInput").ap()
    cst = nc.dram_tensor("cst", [128, 260], BF16, kind="ExternalInput").ap()
    cstf = nc.dram_tensor("cstf", [128, 2], F32, kind="ExternalInput").ap()
    # transposed output [D, num_nodes]; host transposes back
    outT = nc.dram_tensor("outT", [D, num_nodes], F32,
                          kind="ExternalOutput").ap()

    with tile.TileContext(nc) as tc, ExitStack() as ctx:
        cpool = ctx.enter_context(tc.tile_pool(name="cpool", bufs=1))
        slabs = ctx.enter_context(tc.tile_pool(name="slabs", bufs=6))
        smalls = ctx.enter_context(tc.tile_pool(name="smalls", bufs=4))
        dumps = ctx.enter_context(tc.tile_pool(name="dumps", bufs=2))
        walls = ctx.enter_context(tc.tile_pool(name="walls", bufs=3))
        prods = ctx.enter_context(tc.tile_pool(name="prods", bufs=2))
        psA = ctx.enter_context(tc.tile_pool(name="psA", bufs=3, space="PSUM"))
        psY = ctx.enter_context(tc.tile_pool(name="psY", bufs=3, space="PSUM"))

        c_sb = cpool.tile([128, 260], BF16)
        nc.sync.dma_start(c_sb[:], cst)
        cf_sb = cpool.tile([128, 2], F32)
        nc.sync.dma_start(cf_sb[:], cstf)
        id_sb = c_sb[:, 0:128]
        bm_sb = c_sb[:, 128:132]
        wot_sb = c_sb[:, 132:260]
        ob_sb = cf_sb[:, 0:1]
        sfix_sb = cf_sb[:, 1:2]

        st = {}

        def front(b):
            """DMA + 3-way split e-score tile reductions."""
            t0 = b * TPB
            nt = min(TPB, NT - t0)
            gp = (GPS_RT * nt) // TPB
            ac = (ACT_RT * nt) // TPB
            dv = nt - gp - ac

            slab = slabs.tile([128, TPB, TW], FP16, tag="slab")
            nc.sync.dma_start(slab[:, :nt, :], slabd[:, t0:t0 + nt, :])
            s128_sb = smalls.tile([128, 160], BF16, tag="s128")
            nc.sync.dma_start(s128_sb[:], s128[b, :, :])

            eraw = smalls.tile([128, TPB], F32, tag="eraw")
            if nt < TPB:
                nc.vector.memset(eraw[:, nt:], 0.0)
            nc.vector.tensor_reduce(
                eraw[:, :dv], slab[:, :dv, :], axis=AX.X, op=OP.add)
            # GPSIMD halves its tiles (no accum support); DVE finishes
            scr = dumps.tile([128, TPB, TW // 2], FP16, tag="dg")
            for j0 in range(dv, dv + gp, PRODC):
                j1 = min(j0 + PRODC, dv + gp)
                nc.gpsimd.tensor_tensor(
                    out=scr[:, j0:j1, :], in0=slab[:, j0:j1, 0:TW // 2],
                    in1=slab[:, j0:j1, TW // 2:TW], op=OP.add)
            dump_a = dumps.tile([128, TW], FP16, tag="da")
            for j in range(dv + gp, nt):
                nc.scalar.activation(
                    dump_a[:], slab[:, j, :], AF.Copy,
                    accum_out=eraw[:, j:j + 1])
            st[b] = dict(slab=slab, s128=s128_sb, eraw=eraw, nt=nt,
                         scr=scr, dv=dv, gp=gp)

        def stage_fin(b):
            """Second-pass reduce over the GPSIMD half-sums (one stage later
            so GPSIMD has a full iteration of slack)."""
            s = st[b]
            dv, gp = s["dv"], s["gp"]
            if gp:
                nc.vector.tensor_reduce(
                    s["eraw"][:, dv:dv + gp], s["scr"][:, dv:dv + gp, :],
                    axis=AX.X, op=OP.add)

        def stage_es(b):
            s = st[b]
            es = smalls.tile([128, TPB], F32, tag="es")
            nc.vector.transpose(es[:], s["eraw"][:])
            s["es"] = es

        def stage_exp(b):
            s = st[b]
            e3 = smalls.tile([128, TPB], FP16, tag="e3")
            nc.vector.scalar_tensor_tensor(
                e3[:], s["es"][:], ALPHA, s["es"][:], op0=OP.mult, op1=OP.max)
            expt = smalls.tile([128, TPB], BF16, tag="expt")
            sume = smalls.tile([128, 1], F32, tag="sume")
            nc.scalar.activation(expt[:], e3[:], AF.Exp, accum_out=sume[:])
            s.update(expt=expt, sume=sume)

        def stage_w1(b):
            """rcp + coeff (w_sb written; weT read deferred)."""
            s = st[b]
            at_sb = s["s128"][:, 128:160]
            rcp = smalls.tile([128, 1], F32, tag="rcp")
            nc.vector.reciprocal(rcp[:], s["sume"][:])
            w_sb = smalls.tile([128, TPB], BF16, tag="wsm")
            nc.vector.scalar_tensor_tensor(
                w_sb[:], s["expt"][:], rcp[:], at_sb,
                op0=OP.mult, op1=OP.mult)
            s["wsm"] = w_sb

        def stage_weT(b):
            """Transpose coeffs back to edge-major (DVE)."""
            s = st[b]
            weT = smalls.tile([128, TPB], BF16, tag="weT")
            nc.vector.transpose(weT[:], s["wsm"][:])
            s["weT"] = weT

        def stage_wall(b):
            """Blockmask spread (GPSIMD) - emitted after the half-adds; its
            weT dependency is DVE's first op of this iteration, so the
            GPSIMD queue never stalls ahead of the next half-adds."""
            s = st[b]
            wall = walls.tile([128, TPB, 4], BF16, tag="wall")
            nt = s["nt"]
            nc.gpsimd.tensor_mul(
                wall[:, :nt, :],
                s["weT"][:, :nt].unsqueeze(2).broadcast_to((128, nt, 4)),
                bm_sb.unsqueeze(1).broadcast_to((128, nt, 4)))
            s["wall"] = wall

        def stage_prod(b):
            """prod chunks on DVE/GPS (emitted between other DVE ops)."""
            s = st[b]
            slab, nt = s["slab"], s["nt"]
            prod = prods.tile([128, TPB, D], BF16, tag="prod")
            nchunk = (nt + PRODC - 1) // PRODC
            for ci in range(nchunk):
                p0 = ci * PRODC
                p1 = min(p0 + PRODC, nt)
                eng = nc.vector if ci < PROD_DVE_CHUNKS else nc.gpsimd
                eng.tensor_tensor(
                    out=prod[:, p0:p1, :], in0=slab[:, p0:p1, 0:D],
                    in1=slab[:, p0:p1, D:2 * D], op=OP.mult)
            s["prod"] = prod

        def back(b):
            """Weighted aggregation + final linear."""
            s = st.pop(b)
            wall, prod, nt = s["wall"], s["prod"], s["nt"]
            itr_sb = s["s128"][:, 0:D]
            nn = nt * 4
            n0 = b * TPB * 4

            agg_ps = psA.tile([128, TPB * 4], F32, tag="aggps")
            for t in range(nt):
                nc.tensor.matmul(
                    agg_ps[:, 4 * t:4 * t + 4], prod[:, t, :],
                    wall[:, t, :],
                    start=(t == 0), stop=False, skip_group_check=True)
            nc.tensor.matmul(agg_ps[:, :nn], itr_sb[:nn, :],
                             id_sb[:nn, :nn],
                             start=False, stop=True, skip_group_check=True)
            xT_sb = smalls.tile([128, TPB * 4], BF16, tag="xT")
            nc.scalar.activation(xT_sb[:, :nn], agg_ps[:, :nn], AF.Copy,
                                 scale=sfix_sb)

            y_ps = psY.tile([128, TPB * 4], F32, tag="yps")
            nc.tensor.matmul(y_ps[:, :nn], wot_sb, xT_sb[:, :nn],
                             start=True, stop=True)
            yf_sb = smalls.tile([128, TPB * 4], F32, tag="yf")
            nc.scalar.activation(yf_sb[:, :nn], y_ps[:, :nn], AF.Relu,
                                 bias=ob_sb, scale=1.0)
            nc.sync.dma_start(outT[:, n0:n0 + nn], yf_sb[:, :nn])

        # software pipeline; DVE write->read pairs separated by other work.
        # weT(b-4) leads the DVE iteration (its w_sb input is a full
        # iteration old); GPSIMD's wall(b-4) follows the half-adds and only
        # needs that first DVE op, so GPSIMD self-paces and the
        # reduce2<-halfadds<-wall<-weT cross-engine cycle is broken.
        for i in range(NB + 4):
            if i >= 4:
                stage_weT(i - 4)
            if i < NB:
                front(i)
            if i >= 4:
                stage_wall(i - 4)
            if 1 <= i < NB + 1:
                stage_fin(i - 1)
            if 2 <= i < NB + 2:
                stage_exp(i - 2)
            if 1 <= i < NB + 1:
                stage_es(i - 1)
            if 3 <= i < NB + 3:
                stage_w1(i - 3)
            if i >= 4:
                stage_prod(i - 4)
            if i >= 4:
                back(i - 4)

    nc.compile()
    return nc


def host_prep(num_nodes, item_embs, entity_embs, relations_embed, relation_ids,
              adj_mask, fc_w, fc_b, out_w, out_b, rel_dom_probs):
    """Build the per-core input map for one shard (numpy only)."""
    E = num_nodes * K
    NT = E // 128
    NB = (NT + TPB - 1) // TPB
    NPAD = NB * TPB * 4                     # padded node count
    EPAD = NB * TPB * 128                   # padded edge count

    fw = fc_w.astype(np.float32)[0]
    w1, w2, w3 = fw[:D], fw[D:2 * D], fw[2 * D:]
    w23 = (w2 * w3).astype(np.float32)
    sfix = np.where(np.abs(w23) > 1e-30, 1.0 / w23, 0.0).astype(np.float32)

    itm = item_embs.astype(np.float32)
    # maskbias + fc_b + s1(item@w1) per edge
    s1 = itm @ w1
    mb = np.where(adj_mask > 0, np.float32(fc_b[0]),
                  np.float32(NEG_BIG)).astype(np.float32).reshape(
                      num_nodes, K) + s1[:, None]

    # pre-scaled rel|ent + mb col, partition-major [128, NT, TW]
    re = np.zeros((NT, 128, TW), np.float32)
    re[:, :, 0:D] = (relations_embed.reshape(E, D) * w2).reshape(NT, 128, D)
    re[:, :, D:2 * D] = (entity_embs.reshape(E, D) * w3).reshape(NT, 128, D)
    re[:, :, 2 * D] = mb.reshape(-1)[:E].reshape(NT, 128)
    slabd = np.ascontiguousarray(re.transpose(1, 0, 2)).astype(NPF16)

    itmw_p = np.zeros((NPAD, D), np.float32)
    itmw_p[:num_nodes] = itm * w23

    # a_total from the prob table (exact)
    rowsum = rel_dom_probs.astype(np.float32).sum(-1)
    valid = (relation_ids >= 0) & (relation_ids < R)
    at = np.where(valid, rowsum[np.clip(relation_ids, 0, R - 1)],
                  np.float32(0.0)).astype(np.float32).reshape(-1)
    at_p = np.zeros((EPAD,), np.float32)
    at_p[:E] = at

    # s128 pack: [NB, 128, 160] = [item*w2w3(128) | at_mt(32)]
    s128 = np.zeros((NB, 128, 160), np.float32)
    s128[:, :, :D] = itmw_p.reshape(NB, 128, D)
    s128[:, :, D:] = at_p.reshape(NB, TPB, 4, K).transpose(
        0, 2, 1, 3).reshape(NB, 128, K)
    s128 = s128.astype(NPBF)

    cst = np.zeros((128, 260), np.float32)
    cst[:, 0:128] = np.eye(128, dtype=np.float32)
    cst[:, 128:132] = (
        np.arange(128)[:, None] // 32 == np.arange(4)[None, :])
    cst[:, 132:260] = out_w.astype(np.float32).T
    cst = cst.astype(NPBF)
    cstf = np.zeros((128, 2), np.float32)
    cstf[:, 0] = out_b.astype(np.float32)
    cstf[:, 1] = sfix

    return {"slabd": slabd, "s128": s128, "cst": cst, "cstf": cstf}


_NC_CACHE = {}


def _get_nc(num_nodes):
    if num_nodes not in _NC_CACHE:
        _NC_CACHE[num_nodes] = build_kernel(num_nodes)
    return _NC_CACHE[num_nodes]


def kernel(item_embs, entity_embs, relations_embed, relation_ids, adj_mask,
           fc_w, fc_b, out_w, out_b, rel_dom_probs, **_unused):
    item_embs = np.asarray(item_embs)
    entity_embs = np.asarray(entity_embs)
    relations_embed = np.asarray(relations_embed)
    relation_ids = np.asarray(relation_ids)
    adj_mask = np.asarray(adj_mask)
    fc_w = np.asarray(fc_w)
    fc_b = np.asarray(fc_b)
    out_w = np.asarray(out_w)
    out_b = np.asarray(out_b)
    rel_dom_probs = np.asarray(rel_dom_probs)

    n = item_embs.shape[0]
    npc = n // N_CORES
    nc = _get_nc(npc)

    in_maps = []
    for c in range(N_CORES):
        s = slice(c * npc, (c + 1) * npc)
        in_maps.append(host_prep(
            npc, item_embs[s], entity_embs[s], relations_embed[s],
            relation_ids[s], adj_mask[s], fc_w, fc_b, out_w, out_b,
            rel_dom_probs))

    res = run_bass_kernel_spmd(nc, in_maps, list(range(N_CORES)))
    return np.ascontiguousarray(np.concatenate(
        [res.results[c]["outT"] for c in range(N_CORES)],
        axis=1).T).astype(np.float32)


# revision 21
# speedup vs baseline: 1.1887x; 1.1887x over previous
"""Trainium2 Bass kernel for nn_GAT_55344948576482 (GNN message passing).

Sharding: node dimension N=20000 split across 8 NeuronCores (2500 nodes each).
Fully data-parallel SPMD - no collectives. Small weights/tables replicated.

fp16/bf16 pipeline (rel tol 2e-2; measured ~2.3e-3):
  - host packs slab' = [rel*w2 | ent*w3 | maskbias+fc_b+s1 | pad] fp16
    PARTITION-MAJOR (one contiguous DMA segment per partition per block)
  - e-score tile reductions SPLIT 3-WAY by engine rate:
      DVE: one block tensor_reduce over its tile share
      GPSIMD: per-tile half-adds (no accum support), DVE finishes
      ACT: per-tile Copy+accum
  - softmax (no max-subtraction; mask value -300 underflows exp) in
    [(m,t),k] layout via DVE stream_transpose; one exp ACT per block
  - prod' = slab'_rel (.) slab'_ent -> bf16 (3 chunks DVE / 1 GPSIMD)
  - weighted K-sum on PE: aggT += prod'_t.T @ wall_t + residual item*w2w3;
    diag(w2*w3) undone by per-partition ACT scale at PSUM evac
  - final linear transposed, bias+ReLU fused in evac; output [D, N],
    host transposes back
  - software pipeline: front(b) / fin(b-1) / exp(b-2) / es(b-1) /
    w1(b-3) / prod(b-4) / w2(b-4) / back(b-4); same-engine write->read
    pairs separated by independent ops
"""

import sys

sys.path.insert(0, "/opt/trn_rl_repo")

from contextlib import ExitStack

import ml_dtypes
import numpy as np

import concourse.bass as bass
import concourse.tile as tile
from concourse import bacc
from concourse import mybir
from concourse.bass_utils import run_bass_kernel_spmd

F32 = mybir.dt.float32
BF16 = mybir.dt.bfloat16
FP16 = mybir.dt.float16
NPBF = ml_dtypes.bfloat16
NPF16 = np.float16
AF = mybir.ActivationFunctionType
OP = mybir.AluOpType
AX = mybir.AxisListType

N, K, D = 20000, 32, 128
R = 100
N_CORES = 8
ALPHA = 0.2
NEG_BIG = -300.0           # masked-edge bias; exp underflows, fp16-safe
TPB = 32                   # edge-tiles per block (=> 128 nodes per block)
PRODC = 8                  # tiles per prod chunk
TW = 260                   # tile row: 256 rel'|ent' + maskbias + 3 pad

# e-score reduction tile split per 32-tile block (DVE gets the rest)
GPS_RT = 16                # tiles half-added by GPSIMD, then DVE-reduced
ACT_RT = 7                 # tiles via ACT Copy+accum
PROD_DVE_CHUNKS = 3        # of 4 prod chunks per block, how many on DVE


def build_kernel(num_nodes):
    """Build the single-core Bass program for `num_nodes` nodes."""
    E = num_nodes * K
    NT = E // 128                       # number of [128, D] edge tiles
    NB = (NT + TPB - 1) // TPB          # blocks

    nc = bacc.Bacc("TRN2", target_bir_lowering=False, debug=False)

    # pre-scaled rel|ent + maskbias col, partition-major: [128, NT, TW] fp16
    slabd = nc.dram_tensor("slabd", [128, NT, TW], FP16,
                           kind="ExternalInput").ap()
    # per-block pack: [item*w2w3 natural(128) | at_stream(32)]
    s128 = nc.dram_tensor("s128", [NB, 128, 160], BF16,
                          kind="ExternalInput").ap()
    cst = nc.dram_tensor("cst", [128, 260], BF16, kind="ExternalInput").ap()
    cstf = nc.dram_tensor("cstf", [128, 2], F32, kind="ExternalInput").ap()
    # transposed output [D, num_nodes]; host transposes back
    outT = nc.dram_tensor("outT", [D, num_nodes], F32,
                          kind="ExternalOutput").ap()

    with tile.TileContext(nc) as tc, ExitStack() as ctx:
        cpool = ctx.enter_context(tc.tile_pool(name="cpool", bufs=1))
        slabs = ctx.enter_context(tc.tile_pool(name="slabs", bufs=6))
        smalls = ctx.enter_context(tc.tile_pool(name="smalls", bufs=4))
        dumps = ctx.enter_context(tc.tile_pool(name="dumps", bufs=2))
        walls = ctx.enter_context(tc.tile_pool(name="walls", bufs=3))
        prods = ctx.enter_context(tc.tile_pool(name="prods", bufs=2))
        psA = ctx.enter_context(tc.tile_pool(name="psA", bufs=3, space="PSUM"))
        psY = ctx.enter_context(tc.tile_pool(name="psY", bufs=3, space="PSUM"))

        c_sb = cpool.tile([128, 260], BF16)
        nc.sync.dma_start(c_sb[:], cst)
        cf_sb = cpool.tile([128, 2], F32)
        nc.sync.dma_start(cf_sb[:], cstf)
        id_sb = c_sb[:, 0:128]
        bm_sb = c_sb[:, 128:132]
        wot_sb = c_sb[:, 132:260]
        ob_sb = cf_sb[:, 0:1]
        sfix_sb = cf_sb[:, 1:2]

        st = {}

        def front(b):
            """DMA + 3-way split e-score tile reductions."""
            t0 = b * TPB
            nt = min(TPB, NT - t0)
            gp = (GPS_RT * nt) // TPB
            ac = (ACT_RT * nt) // TPB
            dv = nt - gp - ac

            slab = slabs.tile([128, TPB, TW], FP16, tag="slab")
            nc.sync.dma_start(slab[:, :nt, :], slabd[:, t0:t0 + nt, :])
            s128_sb = smalls.tile([128, 160], BF16, tag="s128")
            nc.sync.dma_start(s128_sb[:], s128[b, :, :])

            eraw = smalls.tile([128, TPB], F32, tag="eraw")
            if nt < TPB:
                nc.vector.memset(eraw[:, nt:], 0.0)
            nc.vector.tensor_reduce(
                eraw[:, :dv], slab[:, :dv, :], axis=AX.X, op=OP.add)
            # GPSIMD halves its tiles (no accum support); DVE finishes
            scr = dumps.tile([128, TPB, TW // 2], FP16, tag="dg")
            for j0 in range(dv, dv + gp, PRODC):
                j1 = min(j0 + PRODC, dv + gp)
                nc.gpsimd.tensor_tensor(
                    out=scr[:, j0:j1, :], in0=slab[:, j0:j1, 0:TW // 2],
                    in1=slab[:, j0:j1, TW // 2:TW], op=OP.add)
            dump_a = dumps.tile([128, TW], FP16, tag="da")
            for j in range(dv + gp, nt):
                nc.scalar.activation(
                    dump_a[:], slab[:, j, :], AF.Copy,
                    accum_out=eraw[:, j:j + 1])
            st[b] = dict(slab=slab, s128=s128_sb, eraw=eraw, nt=nt,
                         scr=scr, dv=dv, gp=gp)

        def stage_fin(b):
            """Second-pass reduce over the GPSIMD half-sums (one stage later
            so GPSIMD has a full iteration of slack)."""
            s = st[b]
            dv, gp = s["dv"], s["gp"]
            if gp:
                nc.vector.tensor_reduce(
                    s["eraw"][:, dv:dv + gp], s["scr"][:, dv:dv + gp, :],
                    axis=AX.X, op=OP.add)

        def stage_es(b):
            s = st[b]
            es = smalls.tile([128, TPB], F32, tag="es")
            nc.vector.transpose(es[:], s["eraw"][:])
            s["es"] = es

        def stage_exp(b):
            s = st[b]
            e3 = smalls.tile([128, TPB], FP16, tag="e3")
            nc.vector.scalar_tensor_tensor(
                e3[:], s["es"][:], ALPHA, s["es"][:], op0=OP.mult, op1=OP.max)
            expt = smalls.tile([128, TPB], BF16, tag="expt")
            sume = smalls.tile([128, 1], F32, tag="sume")
            nc.scalar.activation(expt[:], e3[:], AF.Exp, accum_out=sume[:])
            s.update(expt=expt, sume=sume)

        def stage_w1(b):
            """rcp + coeff (w_sb written; weT read deferred)."""
            s = st[b]
            at_sb = s["s128"][:, 128:160]
            rcp = smalls.tile([128, 1], F32, tag="rcp")
            nc.vector.reciprocal(rcp[:], s["sume"][:])
            w_sb = smalls.tile([128, TPB], BF16, tag="wsm")
            nc.vector.scalar_tensor_tensor(
                w_sb[:], s["expt"][:], rcp[:], at_sb,
                op0=OP.mult, op1=OP.mult)
            s["wsm"] = w_sb

        def stage_w2(b):
            """Transpose coeffs back + blockmask spread."""
            s = st[b]
            weT = smalls.tile([128, TPB], BF16, tag="weT")
            nc.vector.transpose(weT[:], s["wsm"][:])
            wall = walls.tile([128, TPB, 4], BF16, tag="wall")
            nt = s["nt"]
            nc.gpsimd.tensor_mul(
                wall[:, :nt, :],
                weT[:, :nt].unsqueeze(2).broadcast_to((128, nt, 4)),
                bm_sb.unsqueeze(1).broadcast_to((128, nt, 4)))
            s["wall"] = wall

        def stage_prod(b):
            """prod chunks on DVE/GPS (emitted between other DVE ops)."""
            s = st[b]
            slab, nt = s["slab"], s["nt"]
            prod = prods.tile([128, TPB, D], BF16, tag="prod")
            nchunk = (nt + PRODC - 1) // PRODC
            for ci in range(nchunk):
                p0 = ci * PRODC
                p1 = min(p0 + PRODC, nt)
                eng = nc.vector if ci < PROD_DVE_CHUNKS else nc.gpsimd
                eng.tensor_tensor(
                    out=prod[:, p0:p1, :], in0=slab[:, p0:p1, 0:D],
                    in1=slab[:, p0:p1, D:2 * D], op=OP.mult)
            s["prod"] = prod

        def back(b):
            """Weighted aggregation + final linear."""
            s = st.pop(b)
            wall, prod, nt = s["wall"], s["prod"], s["nt"]
            itr_sb = s["s128"][:, 0:D]
            nn = nt * 4
            n0 = b * TPB * 4

            agg_ps = psA.tile([128, TPB * 4], F32, tag="aggps")
            for t in range(nt):
                nc.tensor.matmul(
                    agg_ps[:, 4 * t:4 * t + 4], prod[:, t, :],
                    wall[:, t, :],
                    start=(t == 0), stop=False, skip_group_check=True)
            nc.tensor.matmul(agg_ps[:, :nn], itr_sb[:nn, :],
                             id_sb[:nn, :nn],
                             start=False, stop=True, skip_group_check=True)
            xT_sb = smalls.tile([128, TPB * 4], BF16, tag="xT")
            nc.scalar.activation(xT_sb[:, :nn], agg_ps[:, :nn], AF.Copy,
                                 scale=sfix_sb)

            y_ps = psY.tile([128, TPB * 4], F32, tag="yps")
            nc.tensor.matmul(y_ps[:, :nn], wot_sb, xT_sb[:, :nn],
                             start=True, stop=True)
            yf_sb = smalls.tile([128, TPB * 4], F32, tag="yf")
            nc.scalar.activation(yf_sb[:, :nn], y_ps[:, :nn], AF.Relu,
                                 bias=ob_sb, scale=1.0)
            nc.sync.dma_start(outT[:, n0:n0 + nn], yf_sb[:, :nn])

        # software pipeline; DVE write->read pairs separated by other work
        for i in range(NB + 4):
            if i < NB:
                front(i)
            if 1 <= i < NB + 1:
                stage_fin(i - 1)
            if 2 <= i < NB + 2:
                stage_exp(i - 2)
            if 1 <= i < NB + 1:
                stage_es(i - 1)
            if 3 <= i < NB + 3:
                stage_w1(i - 3)
            if i >= 4:
                stage_prod(i - 4)
            if 3 <= i < NB + 3:
                stage_w2(i - 3)
            if i >= 4:
                back(i - 4)

    nc.compile()
    return nc


def host_prep(num_nodes, item_embs, entity_embs, relations_embed, relation_ids,
              adj_mask, fc_w, fc_b, out_w, out_b, rel_dom_probs):
    """Build the per-core input map for one shard (numpy only)."""
    E = num_nodes * K
    NT = E // 128
    NB = (NT + TPB - 1) // TPB
    NPAD = NB * TPB * 4                     # padded node count
    EPAD = NB * TPB * 128                   # padded edge count

    fw = fc_w.astype(np.float32)[0]
    w1, w2, w3 = fw[:D], fw[D:2 * D], fw[2 * D:]
    w23 = (w2 * w3).astype(np.float32)
    sfix = np.where(np.abs(w23) > 1e-30, 1.0 / w23, 0.0).astype(np.float32)

    itm = item_embs.astype(np.float32)
    # maskbias + fc_b + s1(item@w1) per edge
    s1 = itm @ w1
    mb = np.where(adj_mask > 0, np.float32(fc_b[0]),
                  np.float32(NEG_BIG)).astype(np.float32).reshape(
                      num_nodes, K) + s1[:, None]

    # pre-scaled rel|ent + mb col, partition-major [128, NT, TW]
    re = np.zeros((NT, 128, TW), np.float32)
    re[:, :, 0:D] = (relations_embed.reshape(E, D) * w2).reshape(NT, 128, D)
    re[:, :, D:2 * D] = (entity_embs.reshape(E, D) * w3).reshape(NT, 128, D)
    re[:, :, 2 * D] = mb.reshape(-1)[:E].reshape(NT, 128)
    slabd = np.ascontiguousarray(re.transpose(1, 0, 2)).astype(NPF16)

    itmw_p = np.zeros((NPAD, D), np.float32)
    itmw_p[:num_nodes] = itm * w23

    # a_total from the prob table (exact)
    rowsum = rel_dom_probs.astype(np.float32).sum(-1)
    valid = (relation_ids >= 0) & (relation_ids < R)
    at = np.where(valid, rowsum[np.clip(relation_ids, 0, R - 1)],
                  np.float32(0.0)).astype(np.float32).reshape(-1)
    at_p = np.zeros((EPAD,), np.float32)
    at_p[:E] = at

    # s128 pack: [NB, 128, 160] = [item*w2w3(128) | at_mt(32)]
    s128 = np.zeros((NB, 128, 160), np.float32)
    s128[:, :, :D] = itmw_p.reshape(NB, 128, D)
    s128[:, :, D:] = at_p.reshape(NB, TPB, 4, K).transpose(
        0, 2, 1, 3).reshape(NB, 128, K)
    s128 = s128.astype(NPBF)

    cst = np.zeros((128, 260), np.float32)
    cst[:, 0:128] = np.eye(128, dtype=np.float32)
    cst[:, 128:132] = (
        np.arange(128)[:, None] // 32 == np.arange(4)[None, :])
    cst[:, 132:260] = out_w.astype(np.float32).T
    cst = cst.astype(NPBF)
    cstf = np.zeros((128, 2), np.float32)
    cstf[:, 0] = out_b.astype(np.float32)
    cstf[:, 1] = sfix

    return {"slabd": slabd, "s128": s128, "cst": cst, "cstf": cstf}


_NC_CACHE = {}


def _get_nc(num_nodes):
    if num_nodes not in _NC_CACHE:
        _NC_CACHE[num_nodes] = build_kernel(num_nodes)
    return _NC_CACHE[num_nodes]


def kernel(item_embs, entity_embs, relations_embed, relation_ids, adj_mask,
           fc_w, fc_b, out_w, out_b, rel_dom_probs, **_unused):
    item_embs = np.asarray(item_embs)
    entity_embs = np.asarray(entity_embs)
    relations_embed = np.asarray(relations_embed)
    relation_ids = np.asarray(relation_ids)
    adj_mask = np.asarray(adj_mask)
    fc_w = np.asarray(fc_w)
    fc_b = np.asarray(fc_b)
    out_w = np.asarray(out_w)
    out_b = np.asarray(out_b)
    rel_dom_probs = np.asarray(rel_dom_probs)

    n = item_embs.shape[0]
    npc = n // N_CORES
    nc = _get_nc(npc)

    in_maps = []
    for c in range(N_CORES):
        s = slice(c * npc, (c + 1) * npc)
        in_maps.append(host_prep(
            npc, item_embs[s], entity_embs[s], relations_embed[s],
            relation_ids[s], adj_mask[s], fc_w, fc_b, out_w, out_b,
            rel_dom_probs))

    res = run_bass_kernel_spmd(nc, in_maps, list(range(N_CORES)))
    return np.ascontiguousarray(np.concatenate(
        [res.results[c]["outT"] for c in range(N_CORES)],
        axis=1).T).astype(np.float32)
